# revision 14
# baseline (speedup 1.0000x reference)
"""Trainium2 Bass kernel for MQA attention with RMSNorm + positional bias.

Reference computation (per core, seq-sharded over 8 cores):
  xn = rmsnorm(x) * gamma
  q = (xn @ wq) * scale   (16 heads x 128)     k = xn @ wk    v = xn @ wv
  sim = q @ k^T + pos_bias ; masked (non-causal entries := 1e-10)
  attn = softmax(sim); out = (attn @ v, concat heads) @ wo

Sharding: core m owns query rows [256*m, 256*m+256). K/V (shared MQA head)
are computed from own rows and AllGathered. Each core emits its 256 rows of
the final output; the host concatenates.

Precision: q/k projections and q@k^T run as float32r PE matmuls (~11-bit
effective input mantissa, 1 cycle/row at free size >= 256). Measured
end-to-end rel err ~1.3e-2 (threshold 2e-2).

Mask + pos_bias are MULTIPLICATIVE: host sends pbx = exp(pos_bias) with
zeros at masked positions (bf16), so P_unnorm = exp(sim - m) * pbx needs
no full-width add and exp reads PSUM directly. m = rowmax over UNMASKED
sim only: k columns of tiles >= g are zeroed via a host 0/1 gate
(kTA/kTB), and the diagonal tile g is computed from the core's OWN k
(kown, a fixed local address — uniform SPMD program) and masked with
-1e9 in a [128,128] pass.

attn@v runs in [q, dh] orientation (lhsT = P^T tile, rhs = v tile) with V
augmented by a ones column, so the matmul also yields the softmax row sum.
Normalization (x rec) and the reference's masked-value contribution
(e^{-m} * suffix-sum of v, precomputed once as Stail) are applied at the
tiny [128,128] PSUM drain — no full-width normalize / correction passes.

Phase 2 is a 4-stage skewed software pipeline over the 32 (head, qtile)
units: PE retire work for unit i-3 is emitted before unit i's matmuls so
no engine queue head-of-line blocks on a later-emitted producer.
"""

import os

import numpy as np
import ml_dtypes

import concourse.bass as bass
import concourse.mybir as mybir
import concourse.tile as tile
from concourse import bacc, masks
from concourse.bass_utils import run_bass_kernel_spmd

SEQ = 2048
DIM = 2048
H = 16
DH = 128
P = 128
N_CORES = 8
MQ = SEQ // N_CORES      # 256 query rows per core
NQT = MQ // P            # 2 query tiles per core
CD = DIM // P            # 16 contraction chunks
NS = SEQ // P            # 16 seq tiles
LAYW = SEQ + P           # 2048 main cols + 128 diag cols
VW = DH + 4              # v tiles augmented: ones col at DH, zeros pad
NT = H * NQT             # 32 pipeline units
WSPLIT = 1024            # w-mult split point: DVE [0:WSPLIT], Pool rest
SCALE = DH ** -0.5
EPS = 1e-5

FP = mybir.dt.float32
FR = mybir.dt.float32r
BF = mybir.dt.bfloat16
AF = mybir.ActivationFunctionType
ALU = mybir.AluOpType
AX = mybir.AxisListType

last_exec_time_ns = None


def _rms_scale_rows(nc, pool, xt, tag):
    """In-place x *= rsqrt(mean(x^2)+eps) for a [P, DIM] tile."""
    sq = pool.tile([P, DIM], FP, tag="sq_scratch", name="sq_scratch", bufs=1)
    ssq = pool.tile([P, 1], FP, tag=f"ssq{tag}", name=f"ssq{tag}")
    nc.scalar.activation(sq[:], xt[:], AF.Square, accum_out=ssq[:])
    nc.vector.tensor_scalar(ssq[:], ssq[:], 1.0 / DIM, EPS, ALU.mult, ALU.add)
    nc.scalar.sqrt(ssq[:], ssq[:])
    nc.vector.reciprocal(ssq[:], ssq[:])
    nc.vector.tensor_scalar_mul(xt[:], xt[:], ssq[:])


def build():
    nc = bacc.Bacc("TRN2", target_bir_lowering=False, debug=False,
                   num_devices=N_CORES)
    xq_d = nc.dram_tensor("xq", [MQ, DIM], FP, kind="ExternalInput")
    pbx_d = nc.dram_tensor("pbx", [H * MQ, LAYW], BF, kind="ExternalInput")
    md_d = nc.dram_tensor("md", [P, P], BF, kind="ExternalInput")
    lo_d = nc.dram_tensor("lones", [P, P], BF, kind="ExternalInput")
    cnt_d = nc.dram_tensor("cnt", [P, NQT], FP, kind="ExternalInput")
    hz_d = nc.dram_tensor("hz", [P, NQT * NS], BF, kind="ExternalInput")
    gz_d = nc.dram_tensor("gz", [P, NQT * SEQ], BF, kind="ExternalInput")
    g_d = nc.dram_tensor("gamma_t", [P, CD], FP, kind="ExternalInput")
    wq_d = nc.dram_tensor("wq", [DIM, H * DH], FR, kind="ExternalInput")
    wk_d = nc.dram_tensor("wk", [DIM, DH], FR, kind="ExternalInput")
    wv_d = nc.dram_tensor("wv", [DIM, DH], BF, kind="ExternalInput")
    wo_d = nc.dram_tensor("wo", [H * DH, DIM], BF, kind="ExternalInput")
    out_d = nc.dram_tensor("out", [MQ, DIM], FP, kind="ExternalOutput")

    with tile.TileContext(nc) as tc, \
         tc.tile_pool(name="singles", bufs=1) as singles:
        # ---- persistent tiles --------------------------------------------
        ident = singles.tile([P, P], FP, tag="ident", name="ident")
        masks.make_identity(nc, ident[:])
        identb = singles.tile([P, P], BF, tag="identb", name="identb")
        masks.make_identity(nc, identb[:])
        gam = singles.tile([P, CD], FP, tag="gam", name="gam")
        nc.sync.dma_start(out=gam[:], in_=g_d[:])
        md = singles.tile([P, P], BF, tag="md", name="md")
        nc.sync.dma_start(out=md[:], in_=md_d[:])
        cnt = singles.tile([P, NQT], FP, tag="cnt", name="cnt")
        nc.sync.dma_start(out=cnt[:], in_=cnt_d[:])

        qT = singles.tile([P, H, MQ], FR, tag="qT", name="qT")
        kTA = singles.tile([P, SEQ], FR, tag="kTA", name="kTA")
        kTB = singles.tile([P, SEQ], FR, tag="kTB", name="kTB")
        kown_fr = singles.tile([P, MQ], FR, tag="kownfr", name="kown_fr")
        vsb = singles.tile([P, NS, VW], BF, tag="vsb", name="vsb")
        vown = singles.tile([P, NQT, VW], BF, tag="vown", name="vown")
        stail = singles.tile([P, NQT, DH], FP, tag="stail", name="stail")
        oT = singles.tile([P, H, MQ], BF, tag="oT", name="oT")
        wo_sb = singles.tile([P, H, DIM], BF, tag="wo_sb", name="wo_sb")
        for h in range(H):
            nc.sync.dma_start(out=wo_sb[:, h, :],
                              in_=wo_d[h * DH:(h + 1) * DH, :])
        # ones col for the softmax-sum trick
        nc.gpsimd.memset(vsb[:, :, DH:], 0.0)
        nc.gpsimd.memset(vsb[:, :, DH:DH + 1], 1.0)
        nc.gpsimd.memset(vown[:, :, DH:], 0.0)
        nc.gpsimd.memset(vown[:, :, DH:DH + 1], 1.0)

        with tc.tile_pool(name="xnTqp", bufs=1) as xnTqp:
            xnT = xnTqp.tile([P, CD, MQ], FR, tag="xnT", name="xnT")

            # ---- phase 0: own-row xn^T -----------------------------------
            with tc.tile_pool(name="ph0", bufs=2) as ph0, \
                 tc.tile_pool(name="pstr0", bufs=2, space="PSUM") as pstr0:
                xnq = []
                for t in range(NQT):
                    xt = ph0.tile([P, DIM], FP, tag=f"xq{t}", name=f"xq{t}")
                    nc.sync.dma_start(out=xt[:], in_=xq_d[t * P:(t + 1) * P, :])
                    _rms_scale_rows(nc, ph0, xt, f"q{t}")
                    xnq.append(xt)
                for c in range(CD):
                    pt = pstr0.tile([P, MQ], FP, tag="trq", name="trq")
                    for t in range(NQT):
                        nc.tensor.transpose(pt[:, t * P:(t + 1) * P],
                                            xnq[t][:, c * P:(c + 1) * P],
                                            ident[:])
                    nc.vector.tensor_scalar_mul(xnT[:, c, :], pt[:],
                                                gam[:, c:c + 1])

            # ---- phase 1: own-row k/v projection + AllGather -------------
            with tc.tile_pool(name="kvw", bufs=1) as kvwp, \
                 tc.tile_pool(name="dram", bufs=1, space="DRAM") as dramp, \
                 tc.tile_pool(name="psk", bufs=1, space="PSUM") as psk, \
                 tc.tile_pool(name="psv", bufs=1, space="PSUM") as psv, \
                 tc.tile_pool(name="pstv", bufs=2, space="PSUM") as pstv:
                wk_sb = kvwp.tile([P, CD, DH], FR, tag="wk", name="wk_sb")
                wv_sb = kvwp.tile([P, CD, DH], BF, tag="wv", name="wv_sb")
                gz_sb = kvwp.tile([P, NQT, SEQ], BF, tag="gz", name="gz_sb")
                hz_sb = kvwp.tile([P, NQT, NS], BF, tag="hz", name="hz_sb")
                lones = kvwp.tile([P, P], BF, tag="lones", name="lones")
                ones1 = kvwp.tile([1, P], BF, tag="ones1", name="ones1")
                nc.gpsimd.memset(ones1[:], 1.0)
                nc.sync.dma_start(out=lones[:], in_=lo_d[:])
                nc.sync.dma_start(out=hz_sb[:], in_=hz_d[:])
                for t in range(NQT):
                    nc.sync.dma_start(out=gz_sb[:, t, :],
                                      in_=gz_d[:, t * SEQ:(t + 1) * SEQ])
                for c in range(CD):
                    nc.sync.dma_start(out=wk_sb[:, c, :],
                                      in_=wk_d[c * P:(c + 1) * P, :])
                    nc.sync.dma_start(out=wv_sb[:, c, :],
                                      in_=wv_d[c * P:(c + 1) * P, :])
                # k FIRST: kproj -> bounce -> AllGather before any v work,
                # so the collective isn't queued behind vproj on gpsimd
                pk = psk.tile([P, MQ], FP, tag="pk", name="pk")
                for c in range(CD):
                    nc.tensor.matmul(pk[:], lhsT=wk_sb[:, c, :],
                                     rhs=xnT[:, c, :],
                                     start=(c == 0), stop=(c == CD - 1))
                kown = kvwp.tile([P, MQ], FP, tag="kown", name="kown")
                nc.scalar.copy(kown[:], pk[:])
                nc.scalar.copy(kown_fr[:], pk[:])
                k_bounce = dramp.tile([P, MQ], FP, tag="kb", name="k_bounce")
                k_ag = dramp.tile([N_CORES * P, MQ], FP, tag="kag",
                                  name="k_ag", addr_space="Shared")
                v_bounce = dramp.tile([MQ, DH], BF, tag="vb", name="v_bounce")
                v_ag = dramp.tile([SEQ, DH], BF, tag="vag", name="v_ag",
                                  addr_space="Shared")
                nc.gpsimd.dma_start(k_bounce[:], kown[:])
                rg = [list(range(N_CORES))]
                nc.gpsimd.collective_compute(
                    "AllGather", ALU.bypass, replica_groups=rg,
                    ins=[k_bounce[:].opt()], outs=[k_ag[:].opt()])
                kstg = kvwp.tile([P, SEQ], FP, tag="kstg", name="kstg")
                for r in range(N_CORES):
                    nc.scalar.dma_start(
                        out=kstg[:, r * MQ:(r + 1) * MQ],
                        in_=k_ag[r * P:(r + 1) * P, :])
                # gated k copies: kTA (tile 0) / kTB (tile 1)
                nc.vector.tensor_tensor(kTA[:], kstg[:], gz_sb[:, 0, :],
                                        op=ALU.mult)
                nc.vector.tensor_tensor(kTB[:], kstg[:], gz_sb[:, 1, :],
                                        op=ALU.mult)
                # now v: projection, transpose, gather
                xnTb = kvwp.tile([P, CD, MQ], BF, tag="xnTb", name="xnTb")
                for c in range(CD):
                    nc.scalar.copy(xnTb[:, c, :], xnT[:, c, :])
                pv = psv.tile([P, MQ], FP, tag="pv", name="pv")
                for c in range(CD):
                    nc.tensor.matmul(pv[:], lhsT=wv_sb[:, c, :],
                                     rhs=xnTb[:, c, :],
                                     start=(c == 0), stop=(c == CD - 1))
                vTs = kvwp.tile([P, MQ], FP, tag="vTs", name="vTs")
                nc.vector.tensor_copy(vTs[:], pv[:])
                for t in range(NQT):
                    ptv = pstv.tile([P, P], FP, tag="vtr", name="vtr")
                    nc.tensor.transpose(ptv[:], vTs[:, t * P:(t + 1) * P],
                                        ident[:])
                    nc.vector.tensor_copy(vown[:, t, 0:DH], ptv[:])
                for t in range(NQT):
                    nc.gpsimd.dma_start(v_bounce[t * P:(t + 1) * P, :],
                                        vown[:, t, 0:DH])
                nc.gpsimd.collective_compute(
                    "AllGather", ALU.bypass, replica_groups=rg,
                    ins=[v_bounce[:].opt()], outs=[v_ag[:].opt()])
                for s in range(NS):
                    nc.gpsimd.dma_start(out=vsb[:, s, 0:DH],
                                        in_=v_ag[s * P:(s + 1) * P, :])
                # Stail[t] = sum_{j > global row} v_j  (within-diag via lones,
                # full tiles beyond g via hz-selected column sums)
                vsuf = kvwp.tile([1, DH], BF, tag="vsuf", name="vsuf")
                for t in range(NQT):
                    psv1 = pstv.tile([1, DH], FP, tag="psv1", name="psv1")
                    for s in range(NS):
                        nc.tensor.matmul(psv1[:], lhsT=hz_sb[:, t, s:s + 1],
                                         rhs=vsb[:, s, 0:DH],
                                         start=(s == 0), stop=(s == NS - 1))
                    nc.scalar.copy(vsuf[:], psv1[:])
                    pst = pstv.tile([P, DH], FP, tag="pst", name="pst")
                    nc.tensor.matmul(pst[:], lhsT=lones[:],
                                     rhs=vown[:, t, 0:DH],
                                     start=True, stop=False)
                    nc.tensor.matmul(pst[:], lhsT=ones1[:], rhs=vsuf[:],
                                     start=False, stop=True,
                                     skip_group_check=True)
                    nc.vector.tensor_copy(stail[:, t, :], pst[:])

        # ---- phase 2: q proj + attention, 4-stage skewed pipeline --------
            with tc.tile_pool(name="pos", bufs=3) as posp, \
                 tc.tile_pool(name="pp", bufs=4) as ppool, \
                 tc.tile_pool(name="pts", bufs=2) as ptsp, \
                 tc.tile_pool(name="st", bufs=6) as stp, \
                 tc.tile_pool(name="wqp", bufs=24) as wqp, \
                 tc.tile_pool(name="psq", bufs=1, space="PSUM") as psq, \
                 tc.tile_pool(name="ps_sim", bufs=4, space="PSUM") as ps_sim, \
                 tc.tile_pool(name="ps_d", bufs=1, space="PSUM") as ps_d, \
                 tc.tile_pool(name="ps_pt", bufs=1, space="PSUM") as ps_pt, \
                 tc.tile_pool(name="ps_o", bufs=1, space="PSUM") as ps_o:

                def qproj(h):
                    pq = psq.tile([P, MQ], FP, tag="pq", name="pq")
                    for c in range(CD):
                        wt = wqp.tile([P, P], FR, tag="wq", name="wqt")
                        nc.sync.dma_start(
                            out=wt[:],
                            in_=wq_d[c * P:(c + 1) * P, h * DH:(h + 1) * DH])
                        nc.tensor.matmul(pq[:], lhsT=wt[:], rhs=xnT[:, c, :],
                                         start=(c == 0), stop=(c == CD - 1))
                    nc.scalar.copy(qT[:, h, :], pq[:])

                S = {}   # per-unit pipeline state

                def stage_mm(i):
                    h, t = i // NQT, i % NQT
                    kTsel = kTA if t == 0 else kTB
                    qsl = qT[:, h, t * P:(t + 1) * P]
                    pbx_t = posp.tile([P, LAYW], BF, tag="pbx", name="pbx")
                    nc.sync.dma_start(
                        out=pbx_t[:],
                        in_=pbx_d[h * MQ + t * P: h * MQ + (t + 1) * P, :])
                    mxc = stp.tile([P, 5], FP, tag="mxc", name="mxc")
                    psims = []
                    for nk in range(4):
                        psim = ps_sim.tile([P, 512], FP, tag="psim",
                                           name="psim")
                        nc.tensor.matmul(psim[:], lhsT=qsl,
                                         rhs=kTsel[:, nk * 512:(nk + 1) * 512],
                                         start=True, stop=True)
                        nc.vector.tensor_reduce(mxc[:, nk:nk + 1], psim[:],
                                                axis=AX.X, op=ALU.max)
                        psims.append(psim)
                    psimd = ps_d.tile([P, P], FP, tag="psimd", name="psimd")
                    nc.tensor.matmul(psimd[:], lhsT=qsl,
                                     rhs=kown_fr[:, t * P:(t + 1) * P],
                                     start=True, stop=True)
                    sd = stp.tile([P, P], FP, tag="sd", name="sd")
                    nc.vector.scalar_tensor_tensor(
                        sd[:], md[:], -1.0e9, psimd[:],
                        op0=ALU.mult, op1=ALU.add)
                    nc.vector.tensor_reduce(mxc[:, 4:5], sd[:],
                                            axis=AX.X, op=ALU.max)
                    negmax = stp.tile([P, 1], FP, tag="negmax", name="negmax")
                    nc.vector.tensor_reduce(negmax[:], mxc[:], axis=AX.X,
                                            op=ALU.max, negate=True)
                    nc.vector.tensor_scalar(negmax[:], negmax[:], 1.0, 0.0,
                                            ALU.mult, ALU.min)
                    S[i] = {"pbx": pbx_t, "psims": psims, "sd": sd,
                            "negmax": negmax}

                def stage_exp(i):
                    st = S[i]
                    negmax = st["negmax"]
                    pexp = ppool.tile([P, LAYW], BF, tag="pexp", name="pexp")
                    for nk in range(4):
                        nc.scalar.activation(pexp[:, nk * 512:(nk + 1) * 512],
                                             st["psims"][nk][:], AF.Exp,
                                             bias=negmax[:])
                    nc.scalar.activation(pexp[:, SEQ:], st["sd"][:], AF.Exp,
                                         bias=negmax[:])
                    em = stp.tile([P, 1], FP, tag="em", name="em")
                    nc.scalar.activation(em[:], negmax[:], AF.Exp)
                    st["pexp"] = pexp
                    st["em"] = em

                def stage_w(i):
                    st = S[i]
                    pexp, pbx_t = st["pexp"], st["pbx"]
                    nc.vector.tensor_tensor(pexp[:, 0:WSPLIT],
                                            pexp[:, 0:WSPLIT],
                                            pbx_t[:, 0:WSPLIT], op=ALU.mult)
                    nc.gpsimd.tensor_tensor(pexp[:, WSPLIT:],
                                            pexp[:, WSPLIT:],
                                            pbx_t[:, WSPLIT:], op=ALU.mult)

                def stage_pt(i):
                    h, t = i // NQT, i % NQT
                    st = S.pop(i)
                    pexp, em = st["pexp"], st["em"]
                    PT = ptsp.tile([P, NS + 1, P], BF, tag="PT", name="PT")
                    for g0 in range(0, NS, 4):
                        ppt = ps_pt.tile([P, 4 * P], BF, tag="ppt",
                                         name="ppt")
                        for s4 in range(4):
                            nc.tensor.transpose(
                                ppt[:, s4 * P:(s4 + 1) * P],
                                pexp[:, (g0 + s4) * P:(g0 + s4 + 1) * P],
                                identb[:])
                        if (g0 // 4) % 2 == 0:
                            nc.vector.tensor_copy(PT[:, g0:g0 + 4, :], ppt[:])
                        else:
                            nc.scalar.copy(PT[:, g0:g0 + 4, :], ppt[:])
                    pptd = ps_pt.tile([P, 4 * P], BF, tag="ppt", name="pptd")
                    nc.tensor.transpose(pptd[:, 0:P], pexp[:, SEQ:],
                                        identb[:])
                    nc.scalar.copy(PT[:, NS, :], pptd[:, 0:P])
                    po = ps_o.tile([P, VW], FP, tag="po", name="po")
                    for s in range(NS):
                        nc.tensor.matmul(po[:], lhsT=PT[:, s, :],
                                         rhs=vsb[:, s, :],
                                         start=(s == 0), stop=False)
                    nc.tensor.matmul(po[:], lhsT=PT[:, NS, :],
                                     rhs=vown[:, t, :],
                                     start=False, stop=True)
                    ssum = stp.tile([P, 1], FP, tag="ssum", name="ssum")
                    nc.vector.scalar_tensor_tensor(
                        ssum[:], cnt[:, t:t + 1], em[:], po[:, DH:DH + 1],
                        op0=ALU.mult, op1=ALU.add)
                    rec = stp.tile([P, 1], FP, tag="rec", name="rec")
                    nc.vector.reciprocal(rec[:], ssum[:])
                    o1 = stp.tile([P, DH], FP, tag="o1", name="o1")
                    nc.vector.scalar_tensor_tensor(
                        o1[:], stail[:, t, :], em[:], po[:, 0:DH],
                        op0=ALU.mult, op1=ALU.add)
                    o_sb = stp.tile([P, DH], BF, tag="o_sb", name="o_sb")
                    nc.scalar.activation(o_sb[:], o1[:], AF.Copy,
                                         scale=rec[:])
                    ppo = ps_pt.tile([P, 4 * P], BF, tag="ppt", name="ppo")
                    nc.tensor.transpose(ppo[:, 0:P], o_sb[:], identb[:])
                    nc.scalar.copy(oT[:, h, t * P:(t + 1) * P], ppo[:, 0:P])

                QL = 3
                for h in range(QL):
                    qproj(h)
                for i in range(NT + 3):
                    if i - 3 >= 0:
                        stage_pt(i - 3)
                    if i < NT:
                        h, t = i // NQT, i % NQT
                        if t == 0 and h + QL < H:
                            qproj(h + QL)
                        stage_mm(i)
                        stage_exp(i)
                    if 0 <= i - 1 < NT:
                        stage_w(i - 1)

        # ---- phase 4: output projection (bf16, JIT wo streaming) ---------
        with tc.tile_pool(name="osb", bufs=2) as osbp, \
             tc.tile_pool(name="ps_out", bufs=NQT * (DIM // 512),
                          space="PSUM") as ps_out:
            pouts = []
            for t in range(NQT):
                for nk in range(DIM // 512):
                    pouts.append(ps_out.tile([P, 512], FP, tag="pout",
                                             name=f"pout{t}_{nk}"))
            for h in range(H):
                for t in range(NQT):
                    for nk in range(DIM // 512):
                        nc.tensor.matmul(pouts[t * (DIM // 512) + nk][:],
                                         lhsT=oT[:, h, t * P:(t + 1) * P],
                                         rhs=wo_sb[:, h, nk * 512:(nk + 1) * 512],
                                         start=(h == 0), stop=(h == H - 1))
            for t in range(NQT):
                osb = osbp.tile([P, DIM], FP, tag="osb", name="osb")
                for nk in range(DIM // 512):
                    nc.scalar.copy(osb[:, nk * 512:(nk + 1) * 512],
                                   pouts[t * (DIM // 512) + nk][:])
                    nc.sync.dma_start(
                        out=out_d[t * P:(t + 1) * P, nk * 512:(nk + 1) * 512],
                        in_=osb[:, nk * 512:(nk + 1) * 512])

    nc.compile()
    return nc


_NC = None


def kernel(**inputs):
    global _NC, last_exec_time_ns
    x = np.asarray(inputs["x"], dtype=np.float32)[0]          # [SEQ, DIM]
    pos = np.asarray(inputs["pos_bias"], dtype=np.float32)    # [H, SEQ, SEQ]
    gamma = np.asarray(inputs["gamma"], dtype=np.float32)
    wq = np.ascontiguousarray(np.asarray(inputs["wq"], dtype=np.float32))
    wk = np.ascontiguousarray(np.asarray(inputs["wk"], dtype=np.float32))
    wv = np.ascontiguousarray(np.asarray(inputs["wv"], dtype=np.float32))
    wo = np.ascontiguousarray(np.asarray(inputs["wo"], dtype=np.float32))

    if _NC is None:
        _NC = build()

    gamma_t = np.ascontiguousarray(gamma.reshape(CD, P).T)
    x = np.ascontiguousarray(x)
    wqs = np.ascontiguousarray(wq * np.float32(SCALE))
    wv_b = np.ascontiguousarray(wv.astype(ml_dtypes.bfloat16))
    wo_b = np.ascontiguousarray(wo.astype(ml_dtypes.bfloat16))
    EP = np.exp(pos).astype(np.float32)                       # [H, SEQ, SEQ]
    jidx = np.arange(SEQ)
    jl = np.arange(P)
    pidx = np.arange(P)
    triu_strict = (jl[None, :] > pidx[:, None])               # [row, col]
    md = np.ascontiguousarray(triu_strict.astype(ml_dtypes.bfloat16))
    lones = np.ascontiguousarray(triu_strict.T.astype(ml_dtypes.bfloat16))
    in_maps = []
    for m in range(N_CORES):
        q0 = m * MQ
        pbx = np.zeros((H, MQ, LAYW), np.float32)
        gz = np.zeros((P, NQT * SEQ), np.float32)
        hz = np.zeros((P, NQT, NS), np.float32)
        cnt_m = np.zeros((P, NQT), np.float32)
        for t in range(NQT):
            g = 2 * m + t                                     # global q tile
            rs = slice(t * P, (t + 1) * P)
            gq = slice(q0 + t * P, q0 + (t + 1) * P)
            pbx[:, rs, :SEQ] = EP[:, gq, :] * (jidx < P * g)[None, None, :]
            pbx[:, rs, SEQ:] = EP[:, gq, P * g:P * (g + 1)] * (~triu_strict)
            gz[:, t * SEQ:(t + 1) * SEQ] = (jidx < P * g).astype(np.float32)
            for s in range(NS):
                hz[:, t, s] = ((s * P + pidx) >= P * (g + 1)).astype(
                    np.float32)
            cnt_m[:, t] = (SEQ - 1) - (q0 + t * P + pidx)
        in_maps.append({
            "xq": np.ascontiguousarray(x[q0:q0 + MQ]),
            "pbx": np.ascontiguousarray(
                pbx.reshape(H * MQ, LAYW).astype(ml_dtypes.bfloat16)),
            "md": md, "lones": lones,
            "cnt": np.ascontiguousarray(cnt_m),
            "hz": np.ascontiguousarray(
                hz.reshape(P, NQT * NS).astype(ml_dtypes.bfloat16)),
            "gz": np.ascontiguousarray(gz.astype(ml_dtypes.bfloat16)),
            "gamma_t": gamma_t,
            "wq": wqs, "wk": wk, "wv": wv_b, "wo": wo_b,
        })
    trace = os.environ.get("KERNEL_TRACE") == "1"
    res = run_bass_kernel_spmd(_NC, in_maps, core_ids=list(range(N_CORES)),
                               trace=trace)
    last_exec_time_ns = res.exec_time_ns
    out = np.concatenate([res.results[m]["out"] for m in range(N_CORES)],
                         axis=0)[None, ...]
    return out.astype(np.float32)


# revision 15
# speedup vs baseline: 1.1807x; 1.1807x over previous
"""Trainium2 Bass kernel for MQA attention with RMSNorm + positional bias.

Reference computation (per core, seq-sharded over 8 cores):
  xn = rmsnorm(x) * gamma
  q = (xn @ wq) * scale   (16 heads x 128)     k = xn @ wk    v = xn @ wv
  sim = q @ k^T + pos_bias ; masked (non-causal entries := 1e-10)
  attn = softmax(sim); out = (attn @ v, concat heads) @ wo

Sharding: core m owns query rows [256*m, 256*m+256). K/V (shared MQA head)
are computed from own rows and AllGathered. Each core emits its 256 rows of
the final output; the host concatenates.

Precision: q/k projections and q@k^T run as float32r PE matmuls (~11-bit
effective input mantissa, 1 cycle/row at free size >= 256). Measured
end-to-end rel err ~1.3e-2 (threshold 2e-2).

Mask + pos_bias are MULTIPLICATIVE: host sends pbx = exp(pos_bias) with
zeros at masked positions (bf16), so P_unnorm = exp(sim - m) * pbx needs
no full-width add and exp reads PSUM directly. m = rowmax over UNMASKED
sim only: k columns of tiles >= g are zeroed via a host 0/1 gate
(kTA/kTB), and the diagonal tile g is computed from the core's OWN k
(kown, a fixed local address — uniform SPMD program) and masked with
-1e9 in a [128,128] pass.

attn@v runs in [q, dh] orientation (lhsT = P^T tile, rhs = v tile) with V
augmented by a ones column, so the matmul also yields the softmax row sum.
Normalization (x rec) and the reference's masked-value contribution
(e^{-m} * suffix-sum of v, precomputed once as Stail) are applied at the
tiny [128,128] PSUM drain — no full-width normalize / correction passes.

Phase 2 is a 4-stage skewed software pipeline over the 32 (head, qtile)
units: PE retire work for unit i-3 is emitted before unit i's matmuls so
no engine queue head-of-line blocks on a later-emitted producer.
"""

import os

import numpy as np
import ml_dtypes

import concourse.bass as bass
import concourse.mybir as mybir
import concourse.tile as tile
from concourse import bacc, masks
from concourse.bass_utils import run_bass_kernel_spmd

SEQ = 2048
DIM = 2048
H = 16
DH = 128
P = 128
N_CORES = 8
MQ = SEQ // N_CORES      # 256 query rows per core
NQT = MQ // P            # 2 query tiles per core
CD = DIM // P            # 16 contraction chunks
NS = SEQ // P            # 16 seq tiles
LAYW = SEQ + P           # 2048 main cols + 128 diag cols
VW = DH + 4              # v tiles augmented: ones col at DH, zeros pad
NT = H * NQT             # 32 pipeline units
WSPLIT = 1024            # w-mult split point: DVE [0:WSPLIT], Pool rest
SCALE = DH ** -0.5
EPS = 1e-5

FP = mybir.dt.float32
FR = mybir.dt.float32r
BF = mybir.dt.bfloat16
AF = mybir.ActivationFunctionType
ALU = mybir.AluOpType
AX = mybir.AxisListType

last_exec_time_ns = None


def _rms_scale_rows(nc, pool, xt, tag):
    """In-place x *= rsqrt(mean(x^2)+eps) for a [P, DIM] tile."""
    sq = pool.tile([P, DIM], FP, tag="sq_scratch", name="sq_scratch", bufs=1)
    ssq = pool.tile([P, 1], FP, tag=f"ssq{tag}", name=f"ssq{tag}")
    nc.scalar.activation(sq[:], xt[:], AF.Square, accum_out=ssq[:])
    nc.vector.tensor_scalar(ssq[:], ssq[:], 1.0 / DIM, EPS, ALU.mult, ALU.add)
    nc.scalar.sqrt(ssq[:], ssq[:])
    nc.vector.reciprocal(ssq[:], ssq[:])
    nc.vector.tensor_scalar_mul(xt[:], xt[:], ssq[:])


def build():
    nc = bacc.Bacc("TRN2", target_bir_lowering=False, debug=False,
                   num_devices=N_CORES)
    xq_d = nc.dram_tensor("xq", [MQ, DIM], FP, kind="ExternalInput")
    pbx_d = nc.dram_tensor("pbx", [H * MQ, LAYW], BF, kind="ExternalInput")
    md_d = nc.dram_tensor("md", [P, P], BF, kind="ExternalInput")
    lo_d = nc.dram_tensor("lones", [P, P], BF, kind="ExternalInput")
    cnt_d = nc.dram_tensor("cnt", [P, NQT], FP, kind="ExternalInput")
    hz_d = nc.dram_tensor("hz", [P, NQT * NS], BF, kind="ExternalInput")
    gz_d = nc.dram_tensor("gz", [P, NQT * SEQ], BF, kind="ExternalInput")
    g_d = nc.dram_tensor("gamma_t", [P, CD], FP, kind="ExternalInput")
    wq_d = nc.dram_tensor("wq", [DIM, H * DH], FR, kind="ExternalInput")
    wk_d = nc.dram_tensor("wk", [DIM, DH], FR, kind="ExternalInput")
    wv_d = nc.dram_tensor("wv", [DIM, DH], BF, kind="ExternalInput")
    wo_d = nc.dram_tensor("wo", [H * DH, DIM], BF, kind="ExternalInput")
    out_d = nc.dram_tensor("out", [MQ, DIM], FP, kind="ExternalOutput")

    with tile.TileContext(nc) as tc, \
         tc.tile_pool(name="singles", bufs=1) as singles:
        # ---- persistent tiles --------------------------------------------
        ident = singles.tile([P, P], FP, tag="ident", name="ident")
        masks.make_identity(nc, ident[:])
        identb = singles.tile([P, P], BF, tag="identb", name="identb")
        masks.make_identity(nc, identb[:])
        gam = singles.tile([P, CD], FP, tag="gam", name="gam")
        nc.sync.dma_start(out=gam[:], in_=g_d[:])
        md = singles.tile([P, P], BF, tag="md", name="md")
        nc.sync.dma_start(out=md[:], in_=md_d[:])
        cnt = singles.tile([P, NQT], FP, tag="cnt", name="cnt")
        nc.sync.dma_start(out=cnt[:], in_=cnt_d[:])

        qT = singles.tile([P, H, MQ], FR, tag="qT", name="qT")
        kTA = singles.tile([P, SEQ], FR, tag="kTA", name="kTA")
        kTB = singles.tile([P, SEQ], FR, tag="kTB", name="kTB")
        kown_fr = singles.tile([P, MQ], FR, tag="kownfr", name="kown_fr")
        vsb = singles.tile([P, NS, VW], BF, tag="vsb", name="vsb")
        vown = singles.tile([P, NQT, VW], BF, tag="vown", name="vown")
        stail = singles.tile([P, NQT, DH], FP, tag="stail", name="stail")
        oT = singles.tile([P, H, MQ], BF, tag="oT", name="oT")
        wo_sb = singles.tile([P, H, DIM], BF, tag="wo_sb", name="wo_sb")
        # ones col for the softmax-sum trick
        nc.gpsimd.memset(vsb[:, :, DH:], 0.0)
        nc.gpsimd.memset(vsb[:, :, DH:DH + 1], 1.0)
        nc.gpsimd.memset(vown[:, :, DH:], 0.0)
        nc.gpsimd.memset(vown[:, :, DH:DH + 1], 1.0)

        with tc.tile_pool(name="xnTqp", bufs=1) as xnTqp:
            xnT = xnTqp.tile([P, CD, MQ], FR, tag="xnT", name="xnT")

            # ---- phase 0: own-row xn^T -----------------------------------
            with tc.tile_pool(name="ph0", bufs=2) as ph0, \
                 tc.tile_pool(name="pstr0", bufs=2, space="PSUM") as pstr0:
                xnq = []
                for t in range(NQT):
                    xt = ph0.tile([P, DIM], FP, tag=f"xq{t}", name=f"xq{t}")
                    nc.sync.dma_start(out=xt[:], in_=xq_d[t * P:(t + 1) * P, :])
                    _rms_scale_rows(nc, ph0, xt, f"q{t}")
                    xnq.append(xt)
                for c in range(CD):
                    pt = pstr0.tile([P, MQ], FP, tag="trq", name="trq")
                    for t in range(NQT):
                        nc.tensor.transpose(pt[:, t * P:(t + 1) * P],
                                            xnq[t][:, c * P:(c + 1) * P],
                                            ident[:])
                    nc.vector.tensor_scalar_mul(xnT[:, c, :], pt[:],
                                                gam[:, c:c + 1])

            # ---- phase 1: own-row k/v projection + AllGather -------------
            with tc.tile_pool(name="kvw", bufs=1) as kvwp, \
                 tc.tile_pool(name="dram", bufs=1, space="DRAM") as dramp, \
                 tc.tile_pool(name="psk", bufs=1, space="PSUM") as psk, \
                 tc.tile_pool(name="psv", bufs=1, space="PSUM") as psv, \
                 tc.tile_pool(name="pstv", bufs=2, space="PSUM") as pstv:
                wk_sb = kvwp.tile([P, CD, DH], FR, tag="wk", name="wk_sb")
                wv_sb = kvwp.tile([P, CD, DH], BF, tag="wv", name="wv_sb")
                gz_sb = kvwp.tile([P, NQT, SEQ], BF, tag="gz", name="gz_sb")
                hz_sb = kvwp.tile([P, NQT, NS], BF, tag="hz", name="hz_sb")
                lones = kvwp.tile([P, P], BF, tag="lones", name="lones")
                ones1 = kvwp.tile([1, P], BF, tag="ones1", name="ones1")
                nc.gpsimd.memset(ones1[:], 1.0)
                nc.sync.dma_start(out=lones[:], in_=lo_d[:])
                nc.sync.dma_start(out=hz_sb[:], in_=hz_d[:])
                for t in range(NQT):
                    nc.sync.dma_start(out=gz_sb[:, t, :],
                                      in_=gz_d[:, t * SEQ:(t + 1) * SEQ])
                for c in range(CD):
                    nc.sync.dma_start(out=wk_sb[:, c, :],
                                      in_=wk_d[c * P:(c + 1) * P, :])
                    nc.sync.dma_start(out=wv_sb[:, c, :],
                                      in_=wv_d[c * P:(c + 1) * P, :])
                # k FIRST: kproj -> bounce -> AllGather before any v work,
                # so the collective isn't queued behind vproj on gpsimd
                pk = psk.tile([P, MQ], FP, tag="pk", name="pk")
                for c in range(CD):
                    nc.tensor.matmul(pk[:], lhsT=wk_sb[:, c, :],
                                     rhs=xnT[:, c, :],
                                     start=(c == 0), stop=(c == CD - 1))
                kown = kvwp.tile([P, MQ], FP, tag="kown", name="kown")
                nc.scalar.copy(kown[:], pk[:])
                nc.scalar.copy(kown_fr[:], pk[:])
                k_bounce = dramp.tile([P, MQ], FP, tag="kb", name="k_bounce")
                k_ag = dramp.tile([N_CORES * P, MQ], FP, tag="kag",
                                  name="k_ag", addr_space="Shared")
                v_bounce = dramp.tile([MQ, DH], BF, tag="vb", name="v_bounce")
                v_ag = dramp.tile([SEQ, DH], BF, tag="vag", name="v_ag",
                                  addr_space="Shared")
                nc.gpsimd.dma_start(k_bounce[:], kown[:])
                rg = [list(range(N_CORES))]
                nc.gpsimd.collective_compute(
                    "AllGather", ALU.bypass, replica_groups=rg,
                    ins=[k_bounce[:].opt()], outs=[k_ag[:].opt()])
                kstg = kvwp.tile([P, SEQ], FP, tag="kstg", name="kstg")
                for r in range(N_CORES):
                    nc.scalar.dma_start(
                        out=kstg[:, r * MQ:(r + 1) * MQ],
                        in_=k_ag[r * P:(r + 1) * P, :])
                # gated k copies: kTA (tile 0) / kTB (tile 1)
                nc.vector.tensor_tensor(kTA[:], kstg[:], gz_sb[:, 0, :],
                                        op=ALU.mult)
                nc.vector.tensor_tensor(kTB[:], kstg[:], gz_sb[:, 1, :],
                                        op=ALU.mult)
                # now v: projection, transpose, gather
                xnTb = kvwp.tile([P, CD, MQ], BF, tag="xnTb", name="xnTb")
                for c in range(CD):
                    nc.scalar.copy(xnTb[:, c, :], xnT[:, c, :])
                pv = psv.tile([P, MQ], FP, tag="pv", name="pv")
                for c in range(CD):
                    nc.tensor.matmul(pv[:], lhsT=wv_sb[:, c, :],
                                     rhs=xnTb[:, c, :],
                                     start=(c == 0), stop=(c == CD - 1))
                vTs = kvwp.tile([P, MQ], FP, tag="vTs", name="vTs")
                nc.vector.tensor_copy(vTs[:], pv[:])
                for t in range(NQT):
                    ptv = pstv.tile([P, P], FP, tag="vtr", name="vtr")
                    nc.tensor.transpose(ptv[:], vTs[:, t * P:(t + 1) * P],
                                        ident[:])
                    nc.vector.tensor_copy(vown[:, t, 0:DH], ptv[:])
                for t in range(NQT):
                    nc.gpsimd.dma_start(v_bounce[t * P:(t + 1) * P, :],
                                        vown[:, t, 0:DH])
                nc.gpsimd.collective_compute(
                    "AllGather", ALU.bypass, replica_groups=rg,
                    ins=[v_bounce[:].opt()], outs=[v_ag[:].opt()])
                for s in range(NS):
                    nc.gpsimd.dma_start(out=vsb[:, s, 0:DH],
                                        in_=v_ag[s * P:(s + 1) * P, :])
                # Stail[t] = sum_{j > global row} v_j  (within-diag via lones,
                # full tiles beyond g via hz-selected column sums)
                vsuf = kvwp.tile([1, DH], BF, tag="vsuf", name="vsuf")
                for t in range(NQT):
                    psv1 = pstv.tile([1, DH], FP, tag="psv1", name="psv1")
                    for s in range(NS):
                        nc.tensor.matmul(psv1[:], lhsT=hz_sb[:, t, s:s + 1],
                                         rhs=vsb[:, s, 0:DH],
                                         start=(s == 0), stop=(s == NS - 1))
                    nc.scalar.copy(vsuf[:], psv1[:])
                    pst = pstv.tile([P, DH], FP, tag="pst", name="pst")
                    nc.tensor.matmul(pst[:], lhsT=lones[:],
                                     rhs=vown[:, t, 0:DH],
                                     start=True, stop=False)
                    nc.tensor.matmul(pst[:], lhsT=ones1[:], rhs=vsuf[:],
                                     start=False, stop=True,
                                     skip_group_check=True)
                    nc.vector.tensor_copy(stail[:, t, :], pst[:])

        # ---- phase 2: q proj + attention, 4-stage skewed pipeline --------
            with tc.tile_pool(name="pos", bufs=3) as posp, \
                 tc.tile_pool(name="pp", bufs=4) as ppool, \
                 tc.tile_pool(name="pts", bufs=2) as ptsp, \
                 tc.tile_pool(name="st", bufs=6) as stp, \
                 tc.tile_pool(name="wqp", bufs=24) as wqp, \
                 tc.tile_pool(name="psq", bufs=1, space="PSUM") as psq, \
                 tc.tile_pool(name="ps_sim", bufs=4, space="PSUM") as ps_sim, \
                 tc.tile_pool(name="ps_d", bufs=1, space="PSUM") as ps_d, \
                 tc.tile_pool(name="ps_pt", bufs=1, space="PSUM") as ps_pt, \
                 tc.tile_pool(name="ps_o", bufs=1, space="PSUM") as ps_o:

                def qproj(h):
                    pq = psq.tile([P, MQ], FP, tag="pq", name="pq")
                    for c in range(CD):
                        wt = wqp.tile([P, P], FR, tag="wq", name="wqt")
                        nc.sync.dma_start(
                            out=wt[:],
                            in_=wq_d[c * P:(c + 1) * P, h * DH:(h + 1) * DH])
                        nc.tensor.matmul(pq[:], lhsT=wt[:], rhs=xnT[:, c, :],
                                         start=(c == 0), stop=(c == CD - 1))
                    nc.scalar.copy(qT[:, h, :], pq[:])

                S = {}   # per-unit pipeline state

                def stage_mm(i):
                    h, t = i // NQT, i % NQT
                    kTsel = kTA if t == 0 else kTB
                    qsl = qT[:, h, t * P:(t + 1) * P]
                    pbx_t = posp.tile([P, LAYW], BF, tag="pbx", name="pbx")
                    nc.sync.dma_start(
                        out=pbx_t[:],
                        in_=pbx_d[h * MQ + t * P: h * MQ + (t + 1) * P, :])
                    mxc = stp.tile([P, 5], FP, tag="mxc", name="mxc")
                    psims = []
                    for nk in range(4):
                        psim = ps_sim.tile([P, 512], FP, tag="psim",
                                           name="psim")
                        nc.tensor.matmul(psim[:], lhsT=qsl,
                                         rhs=kTsel[:, nk * 512:(nk + 1) * 512],
                                         start=True, stop=True)
                        nc.vector.tensor_reduce(mxc[:, nk:nk + 1], psim[:],
                                                axis=AX.X, op=ALU.max)
                        psims.append(psim)
                    psimd = ps_d.tile([P, P], FP, tag="psimd", name="psimd")
                    nc.tensor.matmul(psimd[:], lhsT=qsl,
                                     rhs=kown_fr[:, t * P:(t + 1) * P],
                                     start=True, stop=True)
                    sd = stp.tile([P, P], FP, tag="sd", name="sd")
                    nc.vector.scalar_tensor_tensor(
                        sd[:], md[:], -1.0e9, psimd[:],
                        op0=ALU.mult, op1=ALU.add)
                    nc.vector.tensor_reduce(mxc[:, 4:5], sd[:],
                                            axis=AX.X, op=ALU.max)
                    negmax = stp.tile([P, 1], FP, tag="negmax", name="negmax")
                    nc.vector.tensor_reduce(negmax[:], mxc[:], axis=AX.X,
                                            op=ALU.max, negate=True)
                    nc.vector.tensor_scalar(negmax[:], negmax[:], 1.0, 0.0,
                                            ALU.mult, ALU.min)
                    S[i] = {"pbx": pbx_t, "psims": psims, "sd": sd,
                            "negmax": negmax}

                def stage_exp(i):
                    st = S[i]
                    negmax = st["negmax"]
                    pexp = ppool.tile([P, LAYW], BF, tag="pexp", name="pexp")
                    for nk in range(4):
                        nc.scalar.activation(pexp[:, nk * 512:(nk + 1) * 512],
                                             st["psims"][nk][:], AF.Exp,
                                             bias=negmax[:])
                    nc.scalar.activation(pexp[:, SEQ:], st["sd"][:], AF.Exp,
                                         bias=negmax[:])
                    em = stp.tile([P, 1], FP, tag="em", name="em")
                    nc.scalar.activation(em[:], negmax[:], AF.Exp)
                    st["pexp"] = pexp
                    st["em"] = em

                def stage_w(i):
                    st = S[i]
                    pexp, pbx_t = st["pexp"], st["pbx"]
                    nc.vector.tensor_tensor(pexp[:, 0:WSPLIT],
                                            pexp[:, 0:WSPLIT],
                                            pbx_t[:, 0:WSPLIT], op=ALU.mult)
                    nc.gpsimd.tensor_tensor(pexp[:, WSPLIT:],
                                            pexp[:, WSPLIT:],
                                            pbx_t[:, WSPLIT:], op=ALU.mult)

                def stage_pt(i):
                    h, t = i // NQT, i % NQT
                    st = S.pop(i)
                    pexp, em = st["pexp"], st["em"]
                    PT = ptsp.tile([P, NS + 1, P], BF, tag="PT", name="PT")
                    for g0 in range(0, NS, 4):
                        ppt = ps_pt.tile([P, 4 * P], BF, tag="ppt",
                                         name="ppt")
                        for s4 in range(4):
                            nc.tensor.transpose(
                                ppt[:, s4 * P:(s4 + 1) * P],
                                pexp[:, (g0 + s4) * P:(g0 + s4 + 1) * P],
                                identb[:])
                        if (g0 // 4) % 2 == 0:
                            nc.vector.tensor_copy(PT[:, g0:g0 + 4, :], ppt[:])
                        else:
                            nc.scalar.copy(PT[:, g0:g0 + 4, :], ppt[:])
                    pptd = ps_pt.tile([P, 4 * P], BF, tag="ppt", name="pptd")
                    nc.tensor.transpose(pptd[:, 0:P], pexp[:, SEQ:],
                                        identb[:])
                    nc.scalar.copy(PT[:, NS, :], pptd[:, 0:P])
                    po = ps_o.tile([P, VW], FP, tag="po", name="po")
                    for s in range(NS):
                        nc.tensor.matmul(po[:], lhsT=PT[:, s, :],
                                         rhs=vsb[:, s, :],
                                         start=(s == 0), stop=False)
                    nc.tensor.matmul(po[:], lhsT=PT[:, NS, :],
                                     rhs=vown[:, t, :],
                                     start=False, stop=True)
                    ssum = stp.tile([P, 1], FP, tag="ssum", name="ssum")
                    nc.vector.scalar_tensor_tensor(
                        ssum[:], cnt[:, t:t + 1], em[:], po[:, DH:DH + 1],
                        op0=ALU.mult, op1=ALU.add)
                    rec = stp.tile([P, 1], FP, tag="rec", name="rec")
                    nc.vector.reciprocal(rec[:], ssum[:])
                    o1 = stp.tile([P, DH], FP, tag="o1", name="o1")
                    nc.vector.scalar_tensor_tensor(
                        o1[:], stail[:, t, :], em[:], po[:, 0:DH],
                        op0=ALU.mult, op1=ALU.add)
                    o_sb = stp.tile([P, DH], BF, tag="o_sb", name="o_sb")
                    nc.scalar.activation(o_sb[:], o1[:], AF.Copy,
                                         scale=rec[:])
                    ppo = ps_pt.tile([P, 4 * P], BF, tag="ppt", name="ppo")
                    nc.tensor.transpose(ppo[:, 0:P], o_sb[:], identb[:])
                    nc.scalar.copy(oT[:, h, t * P:(t + 1) * P], ppo[:, 0:P])

                QL = 3
                for h in range(QL):
                    qproj(h)
                for i in range(NT + 3):
                    if i - 3 >= 0:
                        stage_pt(i - 3)
                    if i < NT:
                        h, t = i // NQT, i % NQT
                        if t == 0 and h + QL < H:
                            qproj(h + QL)
                        stage_mm(i)
                        stage_exp(i)
                    if 0 <= i - 1 < NT:
                        stage_w(i - 1)
                    if i < NT and i % 2 == 1:
                        hw_ = i // 2
                        nc.sync.dma_start(out=wo_sb[:, hw_, :],
                                          in_=wo_d[hw_ * DH:(hw_ + 1) * DH, :])

        # ---- phase 4: output projection (bf16, JIT wo streaming) ---------
        with tc.tile_pool(name="osb", bufs=2) as osbp, \
             tc.tile_pool(name="ps_out", bufs=NQT * (DIM // 512),
                          space="PSUM") as ps_out:
            pouts = []
            for t in range(NQT):
                for nk in range(DIM // 512):
                    pouts.append(ps_out.tile([P, 512], FP, tag="pout",
                                             name=f"pout{t}_{nk}"))
            for h in range(H):
                for t in range(NQT):
                    for nk in range(DIM // 512):
                        nc.tensor.matmul(pouts[t * (DIM // 512) + nk][:],
                                         lhsT=oT[:, h, t * P:(t + 1) * P],
                                         rhs=wo_sb[:, h, nk * 512:(nk + 1) * 512],
                                         start=(h == 0), stop=(h == H - 1))
            for t in range(NQT):
                osb = osbp.tile([P, DIM], FP, tag="osb", name="osb")
                for nk in range(DIM // 512):
                    nc.scalar.copy(osb[:, nk * 512:(nk + 1) * 512],
                                   pouts[t * (DIM // 512) + nk][:])
                    nc.sync.dma_start(
                        out=out_d[t * P:(t + 1) * P, nk * 512:(nk + 1) * 512],
                        in_=osb[:, nk * 512:(nk + 1) * 512])

    nc.compile()
    return nc


_NC = None


def kernel(**inputs):
    global _NC, last_exec_time_ns
    x = np.asarray(inputs["x"], dtype=np.float32)[0]          # [SEQ, DIM]
    pos = np.asarray(inputs["pos_bias"], dtype=np.float32)    # [H, SEQ, SEQ]
    gamma = np.asarray(inputs["gamma"], dtype=np.float32)
    wq = np.ascontiguousarray(np.asarray(inputs["wq"], dtype=np.float32))
    wk = np.ascontiguousarray(np.asarray(inputs["wk"], dtype=np.float32))
    wv = np.ascontiguousarray(np.asarray(inputs["wv"], dtype=np.float32))
    wo = np.ascontiguousarray(np.asarray(inputs["wo"], dtype=np.float32))

    if _NC is None:
        _NC = build()

    gamma_t = np.ascontiguousarray(gamma.reshape(CD, P).T)
    x = np.ascontiguousarray(x)
    wqs = np.ascontiguousarray(wq * np.float32(SCALE))
    wv_b = np.ascontiguousarray(wv.astype(ml_dtypes.bfloat16))
    wo_b = np.ascontiguousarray(wo.astype(ml_dtypes.bfloat16))
    EP = np.exp(pos).astype(np.float32)                       # [H, SEQ, SEQ]
    jidx = np.arange(SEQ)
    jl = np.arange(P)
    pidx = np.arange(P)
    triu_strict = (jl[None, :] > pidx[:, None])               # [row, col]
    md = np.ascontiguousarray(triu_strict.astype(ml_dtypes.bfloat16))
    lones = np.ascontiguousarray(triu_strict.T.astype(ml_dtypes.bfloat16))
    in_maps = []
    for m in range(N_CORES):
        q0 = m * MQ
        pbx = np.zeros((H, MQ, LAYW), np.float32)
        gz = np.zeros((P, NQT * SEQ), np.float32)
        hz = np.zeros((P, NQT, NS), np.float32)
        cnt_m = np.zeros((P, NQT), np.float32)
        for t in range(NQT):
            g = 2 * m + t                                     # global q tile
            rs = slice(t * P, (t + 1) * P)
            gq = slice(q0 + t * P, q0 + (t + 1) * P)
            pbx[:, rs, :SEQ] = EP[:, gq, :] * (jidx < P * g)[None, None, :]
            pbx[:, rs, SEQ:] = EP[:, gq, P * g:P * (g + 1)] * (~triu_strict)
            gz[:, t * SEQ:(t + 1) * SEQ] = (jidx < P * g).astype(np.float32)
            for s in range(NS):
                hz[:, t, s] = ((s * P + pidx) >= P * (g + 1)).astype(
                    np.float32)
            cnt_m[:, t] = (SEQ - 1) - (q0 + t * P + pidx)
        in_maps.append({
            "xq": np.ascontiguousarray(x[q0:q0 + MQ]),
            "pbx": np.ascontiguousarray(
                pbx.reshape(H * MQ, LAYW).astype(ml_dtypes.bfloat16)),
            "md": md, "lones": lones,
            "cnt": np.ascontiguousarray(cnt_m),
            "hz": np.ascontiguousarray(
                hz.reshape(P, NQT * NS).astype(ml_dtypes.bfloat16)),
            "gz": np.ascontiguousarray(gz.astype(ml_dtypes.bfloat16)),
            "gamma_t": gamma_t,
            "wq": wqs, "wk": wk, "wv": wv_b, "wo": wo_b,
        })
    trace = os.environ.get("KERNEL_TRACE") == "1"
    res = run_bass_kernel_spmd(_NC, in_maps, core_ids=list(range(N_CORES)),
                               trace=trace)
    last_exec_time_ns = res.exec_time_ns
    out = np.concatenate([res.results[m]["out"] for m in range(N_CORES)],
                         axis=0)[None, ...]
    return out.astype(np.float32)
